# revision 1
# baseline (speedup 1.0000x reference)
"""CrossCondGPTBase forward on 8 Trainium2 NeuronCores.

Strategy: pure data parallelism over batch (B=8 -> 1 sample per core).
Per core, a 12-layer GPT forward over T=1536 tokens, D=768, H=12 heads.

Device layout: residual stream x kept FEATURE-major ([768, 1536] as six
[128, 1536] SBUF tiles) so every projection matmul uses weights as the
stationary operand (lhsT) with 512-wide streams.  LayerNorm affine params
and the 1/sqrt(hd) query scale are folded into the weight matrices on the
host, so on-device LN is just (x - mean) * rsqrt(var + eps), with the
partition-dim mean/var reductions done as ones-vector matmuls and the
per-token stats broadcast back across partitions with rank-1 matmuls.

Attention computes TRANSPOSED per-head scores sT[j, i] = k_h . q_h so that
softmax's normalizer comes free: exp(sT) tiles (bf16) are the moving
operand of the AV matmul whose stationary operand is token-major V
augmented with a ones column; output row 64 is then the per-token sum of
exp.  Causal masking is a per-tile gpsimd affine_select on the triangular
128x128 block; fully-dead score regions are never computed.  Softmax max-
subtraction is skipped (scores are O(5), exp is safe in fp32).

Matmuls run in fp32r (full PE speed at N>=256, ~1e-4 rounding); p/V/y/Wp/
W2/g are bf16.
"""

import sys
import numpy as np

sys.path.insert(0, "/opt/trn_rl_repo")

import concourse.bass as bass
import concourse.mybir as mybir
import concourse.tile as tile
from concourse import bacc, bass_utils
import ml_dtypes

F32 = mybir.dt.float32
F32R = mybir.dt.float32r
BF16 = mybir.dt.bfloat16
AF = mybir.ActivationFunctionType
ALU = mybir.AluOpType

L, D, H, BS = 12, 768, 12, 512
NM, B = 438, 8
HD = D // H          # 64
T = 3 * BS           # 1536
KC = D // 128        # 6 feature chunks
TT = T // 128        # 12 token tiles
NT = T // 512        # 3 token groups
FF = 4 * D           # 3072
FC = FF // 128       # 24
EPS = 1e-5
N_CORES = 8
N_LAYERS = L         # knob for compile testing
DEBUG = False

_COMPILED = None


def _build():
    nc = bacc.Bacc("TRN2", target_bir_lowering=False, debug=False, num_devices=1)

    # ---- DRAM tensors ----
    d_condT = nc.dram_tensor("condT", [NM, BS], F32R, kind="ExternalInput")
    d_embT = nc.dram_tensor("embT", [D, 2 * BS], F32R, kind="ExternalInput")
    d_pos0 = nc.dram_tensor("pos0", [D, BS], F32, kind="ExternalInput")
    d_condw = nc.dram_tensor("cond_w", [NM, D], F32R, kind="ExternalInput")
    d_condb = nc.dram_tensor("cond_b", [D], F32, kind="ExternalInput")
    d_wq = nc.dram_tensor("Wq", [L, D, D], F32R, kind="ExternalInput")
    d_wk = nc.dram_tensor("Wk", [L, D, D], F32R, kind="ExternalInput")
    d_wv = nc.dram_tensor("Wv", [L, D, H * 65], F32R, kind="ExternalInput")
    d_wp = nc.dram_tensor("Wp", [L, D, D], BF16, kind="ExternalInput")
    d_w1 = nc.dram_tensor("W1", [L, D, FF], F32R, kind="ExternalInput")
    d_w2 = nc.dram_tensor("W2", [L, FF, D], BF16, kind="ExternalInput")
    d_bq = nc.dram_tensor("bq", [L, D], F32, kind="ExternalInput")
    d_bk = nc.dram_tensor("bk", [L, D], F32, kind="ExternalInput")
    d_bv = nc.dram_tensor("bv", [L, H * 65], F32, kind="ExternalInput")
    d_bp = nc.dram_tensor("bp", [L, D], F32, kind="ExternalInput")
    d_b1 = nc.dram_tensor("b1", [L, FF], F32, kind="ExternalInput")
    d_b2 = nc.dram_tensor("b2", [L, D], F32, kind="ExternalInput")
    d_sel2 = nc.dram_tensor("sel2", [2, 128], F32R, kind="ExternalInput")
    d_ones1 = nc.dram_tensor("ones1", [1, 128], F32R, kind="ExternalInput")
    d_onesk = nc.dram_tensor("onesk", [128, 1], F32R, kind="ExternalInput")
    d_out = nc.dram_tensor("out", [D, T], F32, kind="ExternalOutput")
    if DEBUG:
        d_dbg_x0 = nc.dram_tensor("dbg_x0", [D, T], F32R, kind="ExternalOutput")
        d_dbg_xn = nc.dram_tensor("dbg_xn", [D, T], F32R, kind="ExternalOutput")
        d_dbg_q = nc.dram_tensor("dbg_q", [128, T], F32R, kind="ExternalOutput")
        d_dbg_k = nc.dram_tensor("dbg_k", [128, T], F32R, kind="ExternalOutput")
        d_dbg_v = nc.dram_tensor("dbg_v", [128, H * 65], BF16, kind="ExternalOutput")
        d_dbg_sums = nc.dram_tensor("dbg_sums", [H, T], F32R, kind="ExternalOutput")
        d_dbg_y = nc.dram_tensor("dbg_y", [D, T], BF16, kind="ExternalOutput")

    with tile.TileContext(nc) as tc:
        _emit(nc, tc, locals())
    nc.compile()
    return nc


def _emit(nc, tc, d):
    import contextlib

    ctx = contextlib.ExitStack()
    st = ctx.enter_context(tc.tile_pool(name="state", bufs=1))
    qk = ctx.enter_context(tc.tile_pool(name="qk", bufs=1))
    scr = ctx.enter_context(tc.tile_pool(name="scr", bufs=8))
    wp_ = ctx.enter_context(tc.tile_pool(name="wpool", bufs=2))
    ps = ctx.enter_context(tc.tile_pool(name="ps", bufs=2, space="PSUM"))
    ps1 = ctx.enter_context(tc.tile_pool(name="ps1", bufs=1, space="PSUM"))
    ps3 = ctx.enter_context(tc.tile_pool(name="ps3", bufs=3, space="PSUM"))

    # persistent state tiles
    x_t = [st.tile([128, T], F32R, tag=f"x{i}", name=f"x{i}") for i in range(KC)]
    xn_t = [st.tile([128, T], F32R, tag=f"xn{i}", name=f"xn{i}") for i in range(KC)]
    y_t = [st.tile([128, T], BF16, tag=f"y{i}", name=f"y{i}") for i in range(KC)]
    v_t = [st.tile([128, H * 65], BF16, tag=f"v{i}", name=f"v{i}") for i in range(TT)]
    sums = st.tile([H, T], F32R, tag="sums")
    ytmp = st.tile([65, T], F32R, tag="ytmp")
    lnscr1 = st.tile([128, 512], F32R, tag="lnscr1")
    lnscr2 = st.tile([128, 512], F32R, tag="lnscr2")
    sel2_t = st.tile([2, 128], F32R, tag="sel2")
    ones1_t = st.tile([1, 128], F32R, tag="ones1")
    onesk_t = st.tile([128, 1], F32R, tag="onesk")
    eps_t = st.tile([128, 1], F32, tag="eps")
    nc.vector.memset(eps_t, EPS)
    nc.sync.dma_start(out=sel2_t, in_=d["d_sel2"].ap())
    nc.sync.dma_start(out=ones1_t, in_=d["d_ones1"].ap())
    nc.sync.dma_start(out=onesk_t, in_=d["d_onesk"].ap())

    def wtile(shape, dtype):
        return wp_.tile(shape, dtype, tag="w", name="w")

    # ---- init: x = [cond @ cond_w + cond_b + pos | emb(+pos) ] ----
    ct = wtile([128, 4 * BS], F32R)
    nc.sync.dma_start(
        out=ct[:, 0 : 3 * BS].rearrange("p (c n) -> p c n", c=3),
        in_=d["d_condT"].ap()[0:384, :].rearrange("(c p) n -> p c n", p=128),
    )
    nc.sync.dma_start(out=ct[0:54, 3 * BS : 4 * BS], in_=d["d_condT"].ap()[384:438, :])
    cw = wtile([128, 4 * D], F32R)
    nc.sync.dma_start(
        out=cw[:, 0 : 3 * D].rearrange("p (c n) -> p c n", c=3),
        in_=d["d_condw"].ap()[0:384, :].rearrange("(c p) n -> p c n", p=128),
    )
    nc.sync.dma_start(out=cw[0:54, 3 * D : 4 * D], in_=d["d_condw"].ap()[384:438, :])
    cb_t = st.tile([128, KC], F32, tag="condb")
    nc.sync.dma_start(out=cb_t, in_=d["d_condb"].ap().rearrange("(c p) -> p c", p=128))
    for mc in range(KC):
        nc.gpsimd.dma_start(
            out=x_t[mc][:, 0:BS], in_=d["d_pos0"].ap()[mc * 128 : (mc + 1) * 128, :]
        )
        # embeddings (pos already added on host)
        nc.sync.dma_start(
            out=x_t[mc][:, BS:T], in_=d["d_embT"].ap()[mc * 128 : (mc + 1) * 128, :]
        )
    for mc in range(KC):
        pt = ps.tile([128, BS], F32, tag="ps", name="ps")
        for kci in range(4):
            kk = 128 if kci < 3 else 54
            nc.tensor.matmul(
                pt,
                cw[0:kk, kci * D + mc * 128 : kci * D + mc * 128 + 128],
                ct[0:kk, kci * BS : (kci + 1) * BS],
                start=(kci == 0),
                stop=(kci == 3),
            )
        # pos segment 0 was DMA'd into x below; add proj + bias on top
        nc.vector.scalar_tensor_tensor(
            out=x_t[mc][:, 0:BS],
            in0=pt,
            scalar=cb_t[:, mc : mc + 1],
            in1=x_t[mc][:, 0:BS],
            op0=ALU.add,
            op1=ALU.add,
        )

    def layernorm():
        """x -> xn = (x - mean) * rsqrt(var + eps), feature-major."""
        for nt in range(NT):
            s1 = ps.tile([128, 512], F32, tag="ps", name="ps")
            s2 = ps.tile([128, 512], F32, tag="ps", name="ps")
            sl = slice(nt * 512, (nt + 1) * 512)
            for kc in range(KC):
                nc.tensor.matmul(
                    s1[0:1, :], onesk_t, x_t[kc][:, sl],
                    start=(kc == 0), stop=(kc == KC - 1),
                )
            for kc in range(KC):  # xsq scratch in xn tiles, sliced per nt
                nc.scalar.activation(out=xn_t[kc][:, sl], in_=x_t[kc][:, sl],
                                     func=AF.Square)
                nc.tensor.matmul(
                    s2[0:1, :], onesk_t, xn_t[kc][:, sl],
                    start=(kc == 0), stop=(kc == KC - 1),
                )
            # evac per-token sums to SBUF (scaled to means on the lhsT host side)
            nc.scalar.activation(out=lnscr1[0:1, :], in_=s1[0:1, :], func=AF.Identity,
                                 scale=1.0 / D)
            nc.scalar.activation(out=lnscr2[0:1, :], in_=s2[0:1, :], func=AF.Identity,
                                 scale=1.0 / D)
            m_bc = ps.tile([128, 512], F32, tag="ps", name="ps")
            q_bc = ps.tile([128, 512], F32, tag="ps", name="ps")
            nc.tensor.matmul(m_bc, ones1_t, lnscr1[0:1, :], start=True, stop=True)
            nc.tensor.matmul(q_bc, ones1_t, lnscr2[0:1, :], start=True, stop=True)
            # var = E[x^2] - mean^2  -> lnscr1 ; a = rsqrt(var+eps) -> lnscr2
            nc.scalar.activation(out=lnscr1, in_=m_bc, func=AF.Square)
            nc.vector.tensor_tensor(out=lnscr1, in0=q_bc, in1=lnscr1, op=ALU.subtract)
            nc.scalar.activation(out=lnscr2, in_=lnscr1, func=AF.Ln, bias=eps_t)
            nc.scalar.activation(out=lnscr2, in_=lnscr2, func=AF.Exp, scale=-0.5)
            for kc in range(KC):
                nc.vector.tensor_tensor(
                    out=xn_t[kc][:, sl], in0=x_t[kc][:, sl], in1=m_bc, op=ALU.subtract
                )
                nc.vector.tensor_tensor(
                    out=xn_t[kc][:, sl], in0=xn_t[kc][:, sl], in1=lnscr2, op=ALU.mult
                )

    def load_w_cols(dram, l, c0, c1, dtype):
        """Load [D, c0:c1] of a layer weight as [128, KC, c1-c0]."""
        t = wtile([128, KC, c1 - c0], dtype)
        nc.sync.dma_start(
            out=t, in_=dram.ap()[l][:, c0:c1].rearrange("(c p) n -> p c n", p=128)
        )
        return t

    def load_bias(dram, l, n, tag):
        t = st.tile([128, n // 128], F32, tag=tag, name=tag)
        nc.sync.dma_start(
            out=t, in_=dram.ap()[l].rearrange("(c p) -> p c", p=128)
        )
        return t

    for l in range(N_LAYERS):
        if DEBUG and l == 0:
            for mc in range(KC):
                nc.sync.dma_start(
                    out=d["d_dbg_x0"].ap()[mc * 128 : (mc + 1) * 128, :], in_=x_t[mc]
                )
        # ============ LN1 ============
        layernorm()
        if DEBUG and l == 0:
            for mc in range(KC):
                nc.sync.dma_start(
                    out=d["d_dbg_xn"].ap()[mc * 128 : (mc + 1) * 128, :], in_=xn_t[mc]
                )

        # ============ V projection (token-major, padded with ones cols) ====
        wv_a = load_w_cols(d["d_wv"], l, 0, 512, F32R)
        wv_b = load_w_cols(d["d_wv"], l, 512, H * 65, F32R)
        bv_bc = st.tile([128, H * 65], BF16, tag="bvbc")
        nc.gpsimd.dma_start(
            out=bv_bc,
            in_=bass.AP(
                tensor=d["d_bv"].ap().tensor,
                offset=l * H * 65,
                ap=[[0, 128], [1, H * 65]],
            ),
        )
        for tt in range(TT):
            pva = ps3.tile([128, 512], F32, tag="sT", name="pva")
            pvb = ps.tile([128, 268], F32, tag="ps", name="pvb")
            for kc in range(KC):
                lh = xn_t[kc][:, tt * 128 : (tt + 1) * 128]
                nc.tensor.matmul(pva, lh, wv_a[:, kc, :],
                                 start=(kc == 0), stop=(kc == KC - 1))
                nc.tensor.matmul(pvb, lh, wv_b[:, kc, :],
                                 start=(kc == 0), stop=(kc == KC - 1))
            nc.vector.tensor_tensor(out=v_t[tt][:, 0:512], in0=pva,
                                    in1=bv_bc[:, 0:512], op=ALU.add)
            nc.vector.tensor_tensor(out=v_t[tt][:, 512:780], in0=pvb,
                                    in1=bv_bc[:, 512:780], op=ALU.add)
        if DEBUG and l == 0:
            nc.sync.dma_start(out=d["d_dbg_v"].ap(), in_=v_t[0])

        # ============ biases for q/k/p ============
        bq_t = load_bias(d["d_bq"], l, D, "bq")
        bk_t = load_bias(d["d_bk"], l, D, "bk")
        bp_t = load_bias(d["d_bp"], l, D, "bp")

        # ============ per head-pair: project q,k then attention ============
        wq = wk = None
        for hp in range(KC):
            if hp % 3 == 0:  # halves of Wq/Wk covering 3 head-pairs each
                wq = load_w_cols(d["d_wq"], l, hp * 128, (hp + 3) * 128, F32R)
                wk = load_w_cols(d["d_wk"], l, hp * 128, (hp + 3) * 128, F32R)
            q_hp = qk.tile([128, T], F32R, tag="q")
            k_hp = qk.tile([128, T], F32R, tag="k")
            for dst, w, b in ((q_hp, wq, bq_t), (k_hp, wk, bk_t)):
                for nt in range(NT):
                    pt = ps.tile([128, 512], F32, tag="ps", name="ps")
                    hc = hp % 3
                    for kc in range(KC):
                        nc.tensor.matmul(
                            pt,
                            w[:, kc, hc * 128 : (hc + 1) * 128],
                            xn_t[kc][:, nt * 512 : (nt + 1) * 512],
                            start=(kc == 0),
                            stop=(kc == KC - 1),
                        )
                    nc.scalar.activation(
                        out=dst[:, nt * 512 : (nt + 1) * 512], in_=pt,
                        func=AF.Identity, bias=b[:, hp : hp + 1],
                    )
            if DEBUG and l == 0 and hp == 0:
                nc.sync.dma_start(out=d["d_dbg_q"].ap(), in_=q_hp)
                nc.sync.dma_start(out=d["d_dbg_k"].ap(), in_=k_hp)
            for hh in range(2):
                h = 2 * hp + hh
                hs = slice(64 * hh, 64 * hh + 64)
                yT = ps1.tile([65, T], F32, tag="yT", name="yT")
                for kt in range(TT):
                    kr = kt % 4
                    lo = 128 * kr
                    pT = scr.tile([128, NT, 512], BF16, tag="scr", name="pT")
                    for qb in range(NT):
                        sT = ps3.tile([128, 512], F32, tag="sT", name="sT")
                        nc.tensor.matmul(
                            sT[:, lo:512],
                            k_hp[hs, kt * 128 : (kt + 1) * 128],
                            q_hp[hs, qb * 512 + lo : (qb + 1) * 512],
                            start=True,
                            stop=True,
                        )
                        nc.scalar.activation(out=pT[:, qb, lo:512], in_=sT[:, lo:512],
                                             func=AF.Exp)
                    nc.gpsimd.affine_select(
                        out=pT[:, :, lo : lo + 128],
                        in_=pT[:, :, lo : lo + 128],
                        pattern=[[0, NT], [1, 128]],
                        compare_op=ALU.is_ge,
                        fill=0.0,
                        base=0,
                        channel_multiplier=-1,
                    )
                    for qb in range(NT):
                        nc.tensor.matmul(
                            yT[:, qb * 512 + lo : (qb + 1) * 512],
                            v_t[kt][:, h * 65 : h * 65 + 65],
                            pT[:, qb, lo:512],
                            start=(kt == 0),
                            stop=(kt == TT - 1),
                        )
                # evac: y rows (bf16, partition-shifted via DMA) + sums row
                nc.scalar.activation(out=ytmp, in_=yT, func=AF.Identity)
                nc.gpsimd.dma_start(
                    out=y_t[hp][64 * hh : 64 * hh + 64, :], in_=ytmp[0:64, :]
                )
                nc.sync.dma_start(out=sums[h : h + 1, :], in_=ytmp[64:65, :])

        # ============ y normalization: y *= broadcast(1/sums) ============
        if DEBUG and l == 0:
            nc.sync.dma_start(out=d["d_dbg_sums"].ap(), in_=sums)
        nc.scalar.activation(out=sums, in_=sums, func=AF.Ln)
        nc.scalar.activation(out=sums, in_=sums, func=AF.Exp, scale=-1.0)
        for kc in range(KC):
            # stage this head-pair's 1/sums rows at partition base 0 (ytmp is
            # dead by now) so the rank-1 broadcast matmul has a legal rhs base
            nc.sync.dma_start(out=ytmp[0:2, :], in_=sums[2 * kc : 2 * kc + 2, :])
            for nt in range(NT):
                rb = ps.tile([128, 512], F32, tag="ps", name="ps")
                nc.tensor.matmul(
                    rb, sel2_t, ytmp[0:2, nt * 512 : (nt + 1) * 512],
                    start=True, stop=True,
                )
                nc.vector.tensor_tensor(
                    out=y_t[kc][:, nt * 512 : (nt + 1) * 512],
                    in0=y_t[kc][:, nt * 512 : (nt + 1) * 512],
                    in1=rb,
                    op=ALU.mult,
                )

        if DEBUG and l == 0:
            for mc in range(KC):
                nc.sync.dma_start(
                    out=d["d_dbg_y"].ap()[mc * 128 : (mc + 1) * 128, :], in_=y_t[mc]
                )
        # ============ attention out projection + residual ============
        wp = load_w_cols(d["d_wp"], l, 0, D, BF16)
        for mc in range(KC):
            for nt in range(NT):
                pt = ps3.tile([128, 512], F32, tag="sT", name="ps")
                for kc in range(KC):
                    nc.tensor.matmul(
                        pt,
                        wp[:, kc, mc * 128 : (mc + 1) * 128],
                        y_t[kc][:, nt * 512 : (nt + 1) * 512],
                        start=(kc == 0),
                        stop=(kc == KC - 1),
                    )
                nc.vector.scalar_tensor_tensor(
                    out=x_t[mc][:, nt * 512 : (nt + 1) * 512],
                    in0=pt,
                    scalar=bp_t[:, mc : mc + 1],
                    in1=x_t[mc][:, nt * 512 : (nt + 1) * 512],
                    op0=ALU.add,
                    op1=ALU.add,
                )

        # ============ LN2 + MLP ============
        layernorm()
        b1_t = load_bias(d["d_b1"], l, FF, "b1")
        b2_t = load_bias(d["d_b2"], l, D, "b2")
        for nt in range(NT):
            sl = slice(nt * 512, (nt + 1) * 512)
            gt = [scr.tile([128, NT, 512], BF16, tag="scr", name="g") for _ in range(8)]
            for gh in range(8):  # gc groups of 3
                w1 = load_w_cols(d["d_w1"], l, gh * 384, (gh + 1) * 384, F32R)
                for gi in range(3):
                    gc = gh * 3 + gi
                    pt = ps3.tile([128, 512], F32, tag="sT", name="ps")
                    for kc in range(KC):
                        nc.tensor.matmul(
                            pt, w1[:, kc, gi * 128 : (gi + 1) * 128], xn_t[kc][:, sl],
                            start=(kc == 0), stop=(kc == KC - 1),
                        )
                    nc.scalar.activation(
                        out=gt[gc // 3][:, gc % 3, :], in_=pt, func=AF.Gelu,
                        bias=b1_t[:, gc : gc + 1],
                    )
            for mc in range(KC):
                w2 = wp_.tile([128, FC, 128], BF16, tag="w2", name="w2")
                nc.sync.dma_start(
                    out=w2,
                    in_=d["d_w2"].ap()[l][:, mc * 128 : (mc + 1) * 128].rearrange(
                        "(c p) n -> p c n", p=128
                    ),
                )
                pt = ps3.tile([128, 512], F32, tag="sT", name="ps")
                for kc in range(FC):
                    nc.tensor.matmul(
                        pt, w2[:, kc, :], gt[kc // 3][:, kc % 3, :],
                        start=(kc == 0), stop=(kc == FC - 1),
                    )
                nc.vector.scalar_tensor_tensor(
                    out=x_t[mc][:, sl],
                    in0=pt,
                    scalar=b2_t[:, mc : mc + 1],
                    in1=x_t[mc][:, sl],
                    op0=ALU.add,
                    op1=ALU.add,
                )

    # ---- output ----
    for mc in range(KC):
        nc.gpsimd.dma_start(
            out=d["d_out"].ap()[mc * 128 : (mc + 1) * 128, :], in_=x_t[mc]
        )
    ctx.close()


def _prep(inputs):
    """Host-side: fold LN params & scales into weights, gather embeddings."""
    f = lambda a: np.asarray(a, dtype=np.float32)
    idx_up = np.asarray(inputs["idx_up"]).astype(np.int64)
    idx_down = np.asarray(inputs["idx_down"]).astype(np.int64)
    cond = f(inputs["cond"])
    teu, ted = f(inputs["tok_emb_up"]), f(inputs["tok_emb_down"])
    pos = f(inputs["pos_emb"])[0]  # [3*BS, D]
    ln1w, ln1b = f(inputs["ln1_w"]), f(inputs["ln1_b"])
    ln2w, ln2b = f(inputs["ln2_w"]), f(inputs["ln2_b"])
    Wq, bq = f(inputs["Wq"]), f(inputs["bq"])
    Wk, bk = f(inputs["Wk"]), f(inputs["bk"])
    Wv, bv = f(inputs["Wv"]), f(inputs["bv"])
    Wp, bp = f(inputs["Wp"]), f(inputs["bp"])
    W1, b1 = f(inputs["W1"]), f(inputs["b1"])
    W2, b2 = f(inputs["W2"]), f(inputs["b2"])

    sc = 1.0 / np.sqrt(HD)
    Wq_e = np.einsum("ld,ldm->ldm", ln1w, Wq) * sc
    bq_e = (np.einsum("ld,ldm->lm", ln1b, Wq) + bq) * sc
    Wk_e = np.einsum("ld,ldm->ldm", ln1w, Wk)
    bk_e = np.einsum("ld,ldm->lm", ln1b, Wk) + bk
    Wv_e = np.einsum("ld,ldm->ldm", ln1w, Wv)
    bv_e = np.einsum("ld,ldm->lm", ln1b, Wv) + bv
    W1_e = np.einsum("ld,ldm->ldm", ln2w, W1)
    b1_e = np.einsum("ld,ldm->lm", ln2b, W1) + b1

    # pad V weights/bias: per head 64 value cols + 1 ones col (bias=1, W=0)
    Wv_pad = np.zeros((L, D, H * 65), np.float32)
    bv_pad = np.zeros((L, H * 65), np.float32)
    for h in range(H):
        Wv_pad[:, :, h * 65 : h * 65 + 64] = Wv_e[:, :, h * 64 : (h + 1) * 64]
        bv_pad[:, h * 65 : h * 65 + 64] = bv_e[:, h * 64 : (h + 1) * 64]
        bv_pad[:, h * 65 + 64] = 1.0
    sel2 = np.zeros((2, 128), np.float32)
    sel2[0, 0:64] = 1.0
    sel2[1, 64:128] = 1.0

    shared = {
        "cond_w": f(inputs["cond_w"]),
        "cond_b": f(inputs["cond_b"]),
        "pos0": np.ascontiguousarray(pos[:BS].T),
        "Wq": Wq_e, "bq": bq_e, "Wk": Wk_e, "bk": bk_e,
        "Wv": Wv_pad, "bv": bv_pad,
        "Wp": Wp.astype(ml_dtypes.bfloat16), "bp": bp,
        "W1": W1_e, "b1": b1_e,
        "W2": W2.astype(ml_dtypes.bfloat16), "b2": b2,
        "sel2": sel2,
        "ones1": np.ones((1, 128), np.float32),
        "onesk": np.ones((128, 1), np.float32),
    }
    in_maps = []
    for c in range(B):
        emb = np.concatenate(
            [teu[idx_up[c]] + pos[BS : 2 * BS], ted[idx_down[c]] + pos[2 * BS :]], 0
        )  # [1024, D]
        m = dict(shared)
        m["condT"] = np.ascontiguousarray(cond[c].T)
        m["embT"] = np.ascontiguousarray(emb.T)
        in_maps.append(m)
    return in_maps


def _maybe_install_ntff_hook():
    """If tracing was requested via BASS_TRACE, make sure the axon NTFF hook
    module exists (this image's antenv lacks it); never fatal."""
    import os
    if not os.environ.get("BASS_TRACE"):
        return
    try:
        import antenv.axon_hooks  # noqa: F401
        return
    except ImportError:
        pass
    try:
        import types
        import antenv
        sys.path.insert(0, "/root/.axon_site")
        from trn_agent_boot.trn_boot import _ntff_profile_via_ctypes
        hook = _ntff_profile_via_ctypes("/opt/axon/libaxon_pjrt.so")
        mod = types.ModuleType("antenv.axon_hooks")
        mod._hook = hook
        mod.get_axon_ntff_profile_hook = lambda: mod._hook
        mod.set_axon_ntff_profile_hook = lambda h: setattr(mod, "_hook", h)
        sys.modules["antenv.axon_hooks"] = mod
        antenv.axon_hooks = mod
    except Exception:
        import os
        os.environ["BASS_NEVER_TRACE"] = "1"


def kernel(**inputs) -> np.ndarray:
    global _COMPILED
    _maybe_install_ntff_hook()
    if _COMPILED is None:
        _COMPILED = _build()
    nc = _COMPILED
    in_maps = _prep(inputs)
    res = bass_utils.run_bass_kernel_spmd(nc, in_maps, core_ids=list(range(N_CORES)))
    out = np.stack([res.results[c]["out"].T for c in range(B)], 0)
    kernel._last = res
    return out.astype(np.float32)



# revision 19
# speedup vs baseline: 1.1842x; 1.1842x over previous
"""CrossCondGPTBase forward on 8 Trainium2 NeuronCores.

Strategy: pure data parallelism over batch (B=8 -> 1 sample per core).
Per core, a 12-layer GPT forward over T=1536 tokens, D=768, H=12 heads.

v2: full-bf16 GEMM datapath (weights + activations bf16, fp32 PSUM, fp32
residual/LN stats) — bf16 matmuls avoid fp32r's ap<256 4x penalty and halve
stationary-load time.  Causal masking via a DVE broadcast-multiply on the
exp'd probabilities (the old gpsimd affine_select serialized attention).
Partial score matmuls for the kr=2,3 diagonal offsets are merged into
multi-dim-AP single matmuls within one PSUM bank.  PSUM: 3x sT scratch +
yT [65,3,512] accumulator + 2x rotating projection bank = 8 banks.  The
next head-pair's Q/K projection is emitted between scores and AV so the PE
stays busy while ACT runs the exps.  Evacuations ride on DVE (tensor_scalar
with per-partition bias), keeping ACT for exp/gelu/LN-rsqrt only.
"""

import sys
import numpy as np

sys.path.insert(0, "/opt/trn_rl_repo")

import concourse.bass as bass
import concourse.mybir as mybir
import concourse.tile as tile
from concourse import bacc, bass_utils
import ml_dtypes

F32 = mybir.dt.float32
F32R = mybir.dt.float32r
BF16 = mybir.dt.bfloat16
AF = mybir.ActivationFunctionType
ALU = mybir.AluOpType

L, D, H, BS = 12, 768, 12, 512
NM, B = 438, 8
HD = D // H          # 64
T = 3 * BS           # 1536
KC = D // 128        # 6 feature chunks
TT = T // 128        # 12 token tiles
NT = T // 512        # 3 token groups
FF = 4 * D           # 3072
FC = FF // 128       # 24
EPS = 1e-5
N_CORES = 8
N_LAYERS = L         # knob for compile testing
DEBUG = False

_COMPILED = None


def _build():
    nc = bacc.Bacc("TRN2", target_bir_lowering=False, debug=False, num_devices=1)

    # ---- DRAM tensors ----
    d_condT = nc.dram_tensor("condT", [NM, BS], BF16, kind="ExternalInput")
    d_embT = nc.dram_tensor("embT", [D, 2 * BS], F32R, kind="ExternalInput")
    d_pos0 = nc.dram_tensor("pos0", [D, BS], F32, kind="ExternalInput")
    d_condw = nc.dram_tensor("cond_w", [NM, D], BF16, kind="ExternalInput")
    d_condb = nc.dram_tensor("cond_b", [D], F32, kind="ExternalInput")
    d_wq = nc.dram_tensor("Wq", [L, D, D], BF16, kind="ExternalInput")
    d_wk = nc.dram_tensor("Wk", [L, D, D], BF16, kind="ExternalInput")
    d_wv = nc.dram_tensor("Wv", [L, D, H * 65], BF16, kind="ExternalInput")
    d_wp = nc.dram_tensor("Wp", [L, D, D], BF16, kind="ExternalInput")
    d_w1 = nc.dram_tensor("W1", [L, D, FF], BF16, kind="ExternalInput")
    d_w2 = nc.dram_tensor("W2", [L, FF, D], BF16, kind="ExternalInput")
    d_bq = nc.dram_tensor("bq", [L, D], F32, kind="ExternalInput")
    d_bk = nc.dram_tensor("bk", [L, D], F32, kind="ExternalInput")
    d_bv = nc.dram_tensor("bv", [L, H * 65], BF16, kind="ExternalInput")
    d_bp = nc.dram_tensor("bp", [L, D], F32, kind="ExternalInput")
    d_b1 = nc.dram_tensor("b1", [L, FF], F32, kind="ExternalInput")
    d_b2 = nc.dram_tensor("b2", [L, D], F32, kind="ExternalInput")
    d_sel2 = nc.dram_tensor("sel2", [2, 128], BF16, kind="ExternalInput")
    d_ones1 = nc.dram_tensor("ones1", [1, 128], F32R, kind="ExternalInput")
    d_onesk = nc.dram_tensor("onesk", [128, 1], F32R, kind="ExternalInput")
    d_oneskb = nc.dram_tensor("oneskb", [128, 1], BF16, kind="ExternalInput")
    d_mask3 = nc.dram_tensor("mask3", [128, NT * 128], BF16, kind="ExternalInput")
    d_out = nc.dram_tensor("out", [D, T], F32, kind="ExternalOutput")

    with tile.TileContext(nc) as tc:
        _emit(nc, tc, locals())
    nc.compile()
    return nc


def _emit(nc, tc, d):
    import contextlib

    ctx = contextlib.ExitStack()
    st = ctx.enter_context(tc.tile_pool(name="state", bufs=1))
    qk = ctx.enter_context(tc.tile_pool(name="qk", bufs=2))
    scr = ctx.enter_context(tc.tile_pool(name="scr", bufs=4))
    gp = ctx.enter_context(tc.tile_pool(name="gpool", bufs=8))
    wp_ = ctx.enter_context(tc.tile_pool(name="wpool", bufs=1))
    wmlp = ctx.enter_context(tc.tile_pool(name="wmlp", bufs=2))
    psQ = ctx.enter_context(tc.tile_pool(name="psQ", bufs=2, space="PSUM"))
    psY = ctx.enter_context(tc.tile_pool(name="psY", bufs=1, space="PSUM"))
    psS = ctx.enter_context(tc.tile_pool(name="psS", bufs=3, space="PSUM"))

    # persistent state tiles
    x_t = [st.tile([128, T], F32R, tag=f"x{i}", name=f"x{i}") for i in range(KC)]
    xn_t = [st.tile([128, T], BF16, tag=f"xn{i}", name=f"xn{i}") for i in range(KC)]
    y_t = [st.tile([128, T], BF16, tag=f"y{i}", name=f"y{i}") for i in range(KC)]
    v_t = [st.tile([128, H * 65], BF16, tag=f"v{i}", name=f"v{i}") for i in range(TT)]
    sums = st.tile([H, T], BF16, tag="sums")
    rsums = st.tile([H, T], BF16, tag="rsums")
    ytmp = st.tile([65, T], BF16, tag="ytmp")
    lnscr1 = st.tile([128, 512], F32R, tag="lnscr1")
    lnscr2 = st.tile([128, 512], F32R, tag="lnscr2")
    sel2_t = st.tile([2, 128], BF16, tag="sel2")
    ones1_t = st.tile([1, 128], F32R, tag="ones1")
    onesk_t = st.tile([128, 1], F32R, tag="onesk")
    oneskb_t = st.tile([128, 1], BF16, tag="oneskb")
    mask3_t = st.tile([128, NT, 128], BF16, tag="mask3")
    eps_t = st.tile([128, 1], F32, tag="eps")
    nc.vector.memset(eps_t, EPS)
    nc.sync.dma_start(out=sel2_t, in_=d["d_sel2"].ap())
    nc.sync.dma_start(out=ones1_t, in_=d["d_ones1"].ap())
    nc.sync.dma_start(out=onesk_t, in_=d["d_onesk"].ap())
    nc.sync.dma_start(out=oneskb_t, in_=d["d_oneskb"].ap())
    nc.sync.dma_start(
        out=mask3_t, in_=d["d_mask3"].ap().rearrange("p (g n) -> p g n", g=NT)
    )

    # ---- init: x = [cond @ cond_w + cond_b + pos | emb(+pos) ] ----
    ct = wp_.tile([128, 4 * BS], BF16, tag="wv", name="ct")
    nc.sync.dma_start(
        out=ct[:, 0 : 3 * BS].rearrange("p (c n) -> p c n", c=3),
        in_=d["d_condT"].ap()[0:384, :].rearrange("(c p) n -> p c n", p=128),
    )
    nc.sync.dma_start(out=ct[0:54, 3 * BS : 4 * BS], in_=d["d_condT"].ap()[384:438, :])
    cw = wp_.tile([128, 4 * D], BF16, tag="wq", name="cw")
    nc.sync.dma_start(
        out=cw[:, 0 : 3 * D].rearrange("p (c n) -> p c n", c=3),
        in_=d["d_condw"].ap()[0:384, :].rearrange("(c p) n -> p c n", p=128),
    )
    nc.sync.dma_start(out=cw[0:54, 3 * D : 4 * D], in_=d["d_condw"].ap()[384:438, :])
    cb_t = st.tile([128, KC], F32, tag="condb")
    nc.sync.dma_start(out=cb_t, in_=d["d_condb"].ap().rearrange("(c p) -> p c", p=128))
    for mc in range(KC):
        nc.gpsimd.dma_start(
            out=x_t[mc][:, 0:BS], in_=d["d_pos0"].ap()[mc * 128 : (mc + 1) * 128, :]
        )
        # embeddings (pos already added on host)
        nc.sync.dma_start(
            out=x_t[mc][:, BS:T], in_=d["d_embT"].ap()[mc * 128 : (mc + 1) * 128, :]
        )
    for mc in range(KC):
        pt = psQ.tile([128, 512], F32, tag="ps", name="ps")
        for kci in range(4):
            kk = 128 if kci < 3 else 54
            nc.tensor.matmul(
                pt,
                cw[0:kk, kci * D + mc * 128 : kci * D + mc * 128 + 128],
                ct[0:kk, kci * BS : (kci + 1) * BS],
                start=(kci == 0),
                stop=(kci == 3),
            )
        # pos segment 0 was DMA'd into x below; add proj + bias on top
        nc.vector.scalar_tensor_tensor(
            out=x_t[mc][:, 0:BS],
            in0=pt,
            scalar=cb_t[:, mc : mc + 1],
            in1=x_t[mc][:, 0:BS],
            op0=ALU.add,
            op1=ALU.add,
        )

    def layernorm_nt(nt):
        """x -> xn = (x - mean) * rsqrt(var + eps), one 512-token group."""
        s1 = psQ.tile([128, 512], F32, tag="ps", name="s1")
        s2 = psQ.tile([128, 512], F32, tag="ps", name="s2")
        sl = slice(nt * 512, (nt + 1) * 512)
        for kc in range(KC):
            nc.tensor.matmul(
                s1[0:1, :], onesk_t, x_t[kc][:, sl],
                start=(kc == 0), stop=(kc == KC - 1),
            )
        for kc in range(KC):  # x^2 scratch in xn tiles (bf16), per nt
            nc.vector.tensor_tensor(
                out=xn_t[kc][:, sl], in0=x_t[kc][:, sl], in1=x_t[kc][:, sl],
                op=ALU.mult,
            )
            nc.tensor.matmul(
                s2[0:1, :], oneskb_t, xn_t[kc][:, sl],
                start=(kc == 0), stop=(kc == KC - 1),
            )
        # evac per-token sums to SBUF (scaled to means)
        nc.scalar.activation(out=lnscr1[0:1, :], in_=s1[0:1, :], func=AF.Identity,
                             scale=1.0 / D)
        nc.scalar.activation(out=lnscr2[0:1, :], in_=s2[0:1, :], func=AF.Identity,
                             scale=1.0 / D)
        m_bc = psQ.tile([128, 512], F32, tag="ps", name="m_bc")
        q_bc = psQ.tile([128, 512], F32, tag="ps", name="q_bc")
        nc.tensor.matmul(m_bc, ones1_t, lnscr1[0:1, :], start=True, stop=True)
        nc.tensor.matmul(q_bc, ones1_t, lnscr2[0:1, :], start=True, stop=True)
        # var = E[x^2] - mean^2  -> lnscr1 ; a = rsqrt(var+eps) -> lnscr2
        nc.scalar.activation(out=lnscr1, in_=m_bc, func=AF.Square)
        nc.vector.tensor_tensor(out=lnscr1, in0=q_bc, in1=lnscr1, op=ALU.subtract)
        nc.scalar.activation(out=lnscr2, in_=lnscr1, func=AF.Ln, bias=eps_t)
        nc.scalar.activation(out=lnscr2, in_=lnscr2, func=AF.Exp, scale=-0.5)
        for kc in range(KC):
            nc.vector.tensor_tensor(
                out=xn_t[kc][:, sl], in0=x_t[kc][:, sl], in1=m_bc, op=ALU.subtract
            )
            nc.vector.tensor_tensor(
                out=xn_t[kc][:, sl], in0=xn_t[kc][:, sl], in1=lnscr2, op=ALU.mult
            )

    def load_w_cols(pool, dram, l, c0, c1, tag):
        """Load [D, c0:c1] of a layer weight as [128, KC, c1-c0] bf16."""
        t = pool.tile([128, KC, c1 - c0], BF16, tag=tag, name=tag)
        nc.sync.dma_start(
            out=t, in_=dram.ap()[l][:, c0:c1].rearrange("(c p) n -> p c n", p=128)
        )
        return t

    def load_bias(dram, l, n, tag):
        t = st.tile([128, n // 128], F32, tag=tag, name=tag)
        nc.sync.dma_start(
            out=t, in_=dram.ap()[l].rearrange("(c p) -> p c", p=128)
        )
        return t

    def qkproj_chunk(wq, wk, bq_t, bk_t, q_hp, k_hp, hp, nts):
        """Emit Q/K projection matmuls for token groups `nts` of head pair hp."""
        hc = hp
        for dst, w, b in ((q_hp, wq, bq_t), (k_hp, wk, bk_t)):
            for nt in nts:
                pt = psQ.tile([128, 512], F32, tag="ps", name="qk_ps")
                for kc in range(KC):
                    nc.tensor.matmul(
                        pt,
                        w[:, kc, hc * 128 : (hc + 1) * 128],
                        xn_t[kc][:, nt * 512 : (nt + 1) * 512],
                        start=(kc == 0),
                        stop=(kc == KC - 1),
                    )
                nc.vector.tensor_scalar(
                    out=dst[:, nt * 512 : (nt + 1) * 512], in0=pt,
                    scalar1=b[:, hp : hp + 1], scalar2=None, op0=ALU.add,
                )

    for l in range(N_LAYERS):
        # ============ weights/biases up front (DMA overlaps compute) ======
        wv = load_w_cols(wp_, d["d_wv"], l, 0, H * 65, "wv")
        bv_bc = st.tile([128, H * 65], BF16, tag="bvbc")
        nc.gpsimd.dma_start(
            out=bv_bc,
            in_=bass.AP(
                tensor=d["d_bv"].ap().tensor,
                offset=l * H * 65,
                ap=[[0, 128], [1, H * 65]],
            ),
        )
        bq_t = load_bias(d["d_bq"], l, D, "bq")
        bk_t = load_bias(d["d_bk"], l, D, "bk")
        bp_t = load_bias(d["d_bp"], l, D, "bp")
        wq = load_w_cols(wp_, d["d_wq"], l, 0, D, "wq")
        wk = load_w_cols(wp_, d["d_wk"], l, 0, D, "wk")

        # ============ LN1 woven with V projection (token-major) ===========
        for nt in range(NT):
            layernorm_nt(nt)
            for tt in range(4 * nt, 4 * nt + 4):
                pva = psS.tile([128, 512], F32, tag="sT", name="pva")
                pvb = psS.tile([128, 512], F32, tag="sT", name="pvb")
                for kc in range(KC):
                    lh = xn_t[kc][:, tt * 128 : (tt + 1) * 128]
                    nc.tensor.matmul(pva, lh, wv[:, kc, 0:512],
                                     start=(kc == 0), stop=(kc == KC - 1))
                    nc.tensor.matmul(pvb[:, 0:268], lh, wv[:, kc, 512 : H * 65],
                                     start=(kc == 0), stop=(kc == KC - 1))
                nc.vector.tensor_tensor(out=v_t[tt][:, 0:512], in0=pva,
                                        in1=bv_bc[:, 0:512], op=ALU.add)
                nc.vector.tensor_tensor(out=v_t[tt][:, 512:780], in0=pvb[:, 0:268],
                                        in1=bv_bc[:, 512:780], op=ALU.add)

        # ============ attention: per head pair project q,k then attend ====
        qh = [qk.tile([128, T], BF16, tag="q", name=f"q{hp}") for hp in range(KC)]
        kh = [qk.tile([128, T], BF16, tag="k", name=f"k{hp}") for hp in range(KC)]
        qkproj_chunk(wq, wk, bq_t, bk_t, qh[0], kh[0], 0, range(NT))
        for hp in range(KC):
            q_hp, k_hp = qh[hp], kh[hp]
            yT = psY.tile([65, NT, 512], F32, tag="yT", name="yT")
            for hh in range(2):
                h = 2 * hp + hh
                hs = slice(64 * hh, 64 * hh + 64)
                q3 = q_hp[hs, :].rearrange("p (g n) -> p g n", g=NT)
                pT = [None] * TT

                def scores_one(kt):
                    kr = kt % 4
                    lo = 128 * kr
                    w = 512 - lo
                    pT[kt] = scr.tile([128, NT, 512], BF16, tag="pT", bufs=4,
                                      name="pT")
                    kst = k_hp[hs, kt * 128 : (kt + 1) * 128]
                    if kr == 3:
                        sT = psS.tile([128, 512], F32, tag="sT", name="sT")
                        nc.tensor.matmul(
                            sT[:, 0 : NT * w].rearrange("p (g n) -> p g n", g=NT),
                            kst, q3[:, :, lo:512], start=True, stop=True,
                        )
                        nc.scalar.activation(
                            out=pT[kt][:, :, lo:512],
                            in_=sT[:, 0 : NT * w].rearrange("p (g n) -> p g n", g=NT),
                            func=AF.Exp,
                        )
                    elif kr == 2:
                        sT = psS.tile([128, 512], F32, tag="sT", name="sT")
                        nc.tensor.matmul(
                            sT.rearrange("p (g n) -> p g n", g=2),
                            kst, q3[:, 0:2, lo:512], start=True, stop=True,
                        )
                        sT2 = psS.tile([128, 512], F32, tag="sT", name="sT")
                        nc.tensor.matmul(
                            sT2[:, 0:w], kst, q_hp[hs, 2 * 512 + lo : 3 * 512],
                            start=True, stop=True,
                        )
                        nc.scalar.activation(
                            out=pT[kt][:, 0:2, lo:512],
                            in_=sT.rearrange("p (g n) -> p g n", g=2),
                            func=AF.Exp,
                        )
                        nc.scalar.activation(
                            out=pT[kt][:, 2, lo:512], in_=sT2[:, 0:w], func=AF.Exp,
                        )
                    else:
                        for qb in range(NT):
                            sT = psS.tile([128, 512], F32, tag="sT", name="sT")
                            nc.tensor.matmul(
                                sT[:, 0:w], kst,
                                q_hp[hs, qb * 512 + lo : (qb + 1) * 512],
                                start=True, stop=True,
                            )
                            nc.scalar.activation(
                                out=pT[kt][:, qb, lo:512], in_=sT[:, 0:w],
                                func=AF.Exp,
                            )
                    nc.vector.tensor_tensor(
                        out=pT[kt][:, :, lo : lo + 128],
                        in0=pT[kt][:, :, lo : lo + 128],
                        in1=mask3_t,
                        op=ALU.mult,
                    )

                def av_one(kt):
                    lo = 128 * (kt % 4)
                    for qb in range(NT):
                        nc.tensor.matmul(
                            yT[:, qb, lo:512],
                            v_t[kt][:, h * 65 : h * 65 + 65],
                            pT[kt][:, qb, lo:512],
                            start=(kt == 0),
                            stop=(kt == TT - 1),
                        )

                # kt groups of 3: scores(g) -> PE filler -> AV(g) so the PE
                # streams QK projections of the next pair while ACT exps run.
                filler = []
                if hp + 1 < KC:
                    nxt = [hh] if hh == 0 else [1, 2]
                    filler = [(qh[hp + 1], wq, bq_t), (kh[hp + 1], wk, bk_t)]
                    filler = [(dst, w_, b_, nt) for nt in nxt
                              for (dst, w_, b_) in filler]
                fi = 0
                for grp in range(4):
                    kts = range(3 * grp, 3 * grp + 3)
                    for kt in kts:
                        scores_one(kt)
                    # one QK projection chain as PE filler per group
                    if fi < len(filler):
                        dst, w_, b_, nt = filler[fi]
                        fi += 1
                        pt = psQ.tile([128, 512], F32, tag="ps", name="qk_ps")
                        for kc in range(KC):
                            nc.tensor.matmul(
                                pt,
                                w_[:, kc, (hp + 1) * 128 : (hp + 2) * 128],
                                xn_t[kc][:, nt * 512 : (nt + 1) * 512],
                                start=(kc == 0),
                                stop=(kc == KC - 1),
                            )
                        nc.vector.tensor_scalar_add(
                            out=dst[:, nt * 512 : (nt + 1) * 512], in0=pt,
                            scalar1=b_[:, hp + 1 : hp + 2],
                        )
                    for kt in kts:
                        av_one(kt)
                while fi < len(filler):
                    dst, w_, b_, nt = filler[fi]
                    fi += 1
                    pt = psQ.tile([128, 512], F32, tag="ps", name="qk_ps")
                    for kc in range(KC):
                        nc.tensor.matmul(
                            pt,
                            w_[:, kc, (hp + 1) * 128 : (hp + 2) * 128],
                            xn_t[kc][:, nt * 512 : (nt + 1) * 512],
                            start=(kc == 0),
                            stop=(kc == KC - 1),
                        )
                    nc.vector.tensor_scalar_add(
                        out=dst[:, nt * 512 : (nt + 1) * 512], in0=pt,
                        scalar1=b_[:, hp + 1 : hp + 2],
                    )
                # evac: y rows + sums row (bf16); partition-shift via DMA
                nc.vector.tensor_scalar_mul(
                    out=ytmp, in0=yT.rearrange("p g n -> p (g n)"), scalar1=1.0
                )
                nc.sync.dma_start(
                    out=y_t[hp][64 * hh : 64 * hh + 64, :], in_=ytmp[0:64, :]
                )
                nc.sync.dma_start(out=sums[h : h + 1, :], in_=ytmp[64:65, :])

        # ====== per-nt: y-norm -> proj -> LN2 -> fc1 -> fc2 pipeline ======
        with nc.allow_low_precision("1/sums normalization tolerates bf16"):
            nc.vector.reciprocal(out=rsums, in_=sums)
        wp = load_w_cols(wp_, d["d_wp"], l, 0, D, "wv")
        b1_t = load_bias(d["d_b1"], l, FF, "b1")
        b2_t = load_bias(d["d_b2"], l, D, "b2")
        for nt in range(NT):
            sl = slice(nt * 512, (nt + 1) * 512)
            for kc in range(KC):
                rstage = scr.tile([2, 512], BF16, tag="rstage", bufs=2,
                                  name="rstage")
                nc.sync.dma_start(
                    out=rstage,
                    in_=rsums[2 * kc : 2 * kc + 2, nt * 512 : (nt + 1) * 512],
                )
                rb = psQ.tile([128, 512], F32, tag="ps", name="rb")
                nc.tensor.matmul(rb, sel2_t, rstage, start=True, stop=True)
                nc.vector.tensor_tensor(
                    out=y_t[kc][:, sl], in0=y_t[kc][:, sl], in1=rb, op=ALU.mult,
                )
            for mc in range(KC):
                pt = psQ.tile([128, 512], F32, tag="ps", name="pj_ps")
                for kc in range(KC):
                    nc.tensor.matmul(
                        pt,
                        wp[:, kc, mc * 128 : (mc + 1) * 128],
                        y_t[kc][:, sl],
                        start=(kc == 0),
                        stop=(kc == KC - 1),
                    )
                nc.vector.scalar_tensor_tensor(
                    out=x_t[mc][:, sl],
                    in0=pt,
                    scalar=bp_t[:, mc : mc + 1],
                    in1=x_t[mc][:, sl],
                    op0=ALU.add,
                    op1=ALU.add,
                )
            layernorm_nt(nt)
            gt = [gp.tile([128, NT, 512], BF16, tag="g", name="g") for _ in range(8)]
            for gh in range(8):  # gc groups of 3
                w1 = load_w_cols(wmlp, d["d_w1"], l, gh * 384, (gh + 1) * 384, "w1")
                for gi in range(3):
                    gc = gh * 3 + gi
                    pt = psQ.tile([128, 512], F32, tag="ps", name="g_ps")
                    for kc in range(KC):
                        nc.tensor.matmul(
                            pt, w1[:, kc, gi * 128 : (gi + 1) * 128], xn_t[kc][:, sl],
                            start=(kc == 0), stop=(kc == KC - 1),
                        )
                    nc.scalar.activation(
                        out=gt[gh][:, gi, :], in_=pt, func=AF.Gelu,
                        bias=b1_t[:, gc : gc + 1],
                    )
            for mc in range(KC):
                w2 = wmlp.tile([128, FC, 128], BF16, tag="w2", name="w2")
                nc.sync.dma_start(
                    out=w2,
                    in_=d["d_w2"].ap()[l][:, mc * 128 : (mc + 1) * 128].rearrange(
                        "(c p) n -> p c n", p=128
                    ),
                )
                pt = psQ.tile([128, 512], F32, tag="ps", name="f2_ps")
                for kc in range(FC):
                    nc.tensor.matmul(
                        pt, w2[:, kc, :], gt[kc // 3][:, kc % 3, :],
                        start=(kc == 0), stop=(kc == FC - 1),
                    )
                nc.vector.scalar_tensor_tensor(
                    out=x_t[mc][:, sl],
                    in0=pt,
                    scalar=b2_t[:, mc : mc + 1],
                    in1=x_t[mc][:, sl],
                    op0=ALU.add,
                    op1=ALU.add,
                )

    # ---- output ----
    for mc in range(KC):
        nc.gpsimd.dma_start(
            out=d["d_out"].ap()[mc * 128 : (mc + 1) * 128, :], in_=x_t[mc]
        )
    ctx.close()


def _prep(inputs):
    """Host-side: fold LN params & scales into weights, gather embeddings."""
    f = lambda a: np.asarray(a, dtype=np.float32)
    bf = lambda a: np.asarray(a, dtype=np.float32).astype(ml_dtypes.bfloat16)
    idx_up = np.asarray(inputs["idx_up"]).astype(np.int64)
    idx_down = np.asarray(inputs["idx_down"]).astype(np.int64)
    cond = f(inputs["cond"])
    teu, ted = f(inputs["tok_emb_up"]), f(inputs["tok_emb_down"])
    pos = f(inputs["pos_emb"])[0]  # [3*BS, D]
    ln1w, ln1b = f(inputs["ln1_w"]), f(inputs["ln1_b"])
    ln2w, ln2b = f(inputs["ln2_w"]), f(inputs["ln2_b"])
    Wq, bq = f(inputs["Wq"]), f(inputs["bq"])
    Wk, bk = f(inputs["Wk"]), f(inputs["bk"])
    Wv, bv = f(inputs["Wv"]), f(inputs["bv"])
    Wp, bp = f(inputs["Wp"]), f(inputs["bp"])
    W1, b1 = f(inputs["W1"]), f(inputs["b1"])
    W2, b2 = f(inputs["W2"]), f(inputs["b2"])

    sc = 1.0 / np.sqrt(HD)
    Wq_e = np.einsum("ld,ldm->ldm", ln1w, Wq) * sc
    bq_e = (np.einsum("ld,ldm->lm", ln1b, Wq) + bq) * sc
    Wk_e = np.einsum("ld,ldm->ldm", ln1w, Wk)
    bk_e = np.einsum("ld,ldm->lm", ln1b, Wk) + bk
    Wv_e = np.einsum("ld,ldm->ldm", ln1w, Wv)
    bv_e = np.einsum("ld,ldm->lm", ln1b, Wv) + bv
    W1_e = np.einsum("ld,ldm->ldm", ln2w, W1)
    b1_e = np.einsum("ld,ldm->lm", ln2b, W1) + b1

    # pad V weights/bias: per head 64 value cols + 1 ones col (bias=1, W=0)
    Wv_pad = np.zeros((L, D, H * 65), np.float32)
    bv_pad = np.zeros((L, H * 65), np.float32)
    for h in range(H):
        Wv_pad[:, :, h * 65 : h * 65 + 64] = Wv_e[:, :, h * 64 : (h + 1) * 64]
        bv_pad[:, h * 65 : h * 65 + 64] = bv_e[:, h * 64 : (h + 1) * 64]
        bv_pad[:, h * 65 + 64] = 1.0
    sel2 = np.zeros((2, 128), np.float32)
    sel2[0, 0:64] = 1.0
    sel2[1, 64:128] = 1.0
    # per-128-block causal mask, keep col >= row, replicated x3
    m1 = (np.arange(128)[None, :] >= np.arange(128)[:, None]).astype(np.float32)
    mask3 = np.tile(m1, (1, NT))

    shared = {
        "cond_w": bf(inputs["cond_w"]),
        "cond_b": f(inputs["cond_b"]),
        "pos0": np.ascontiguousarray(pos[:BS].T),
        "Wq": bf(Wq_e), "bq": bq_e, "Wk": bf(Wk_e), "bk": bk_e,
        "Wv": bf(Wv_pad), "bv": bv_pad.astype(ml_dtypes.bfloat16),
        "Wp": bf(Wp), "bp": bp,
        "W1": bf(W1_e), "b1": b1_e,
        "W2": bf(W2), "b2": b2,
        "sel2": sel2.astype(ml_dtypes.bfloat16),
        "ones1": np.ones((1, 128), np.float32),
        "onesk": np.ones((128, 1), np.float32),
        "oneskb": np.ones((128, 1), ml_dtypes.bfloat16),
        "mask3": mask3.astype(ml_dtypes.bfloat16),
    }
    in_maps = []
    for c in range(B):
        emb = np.concatenate(
            [teu[idx_up[c]] + pos[BS : 2 * BS], ted[idx_down[c]] + pos[2 * BS :]], 0
        )  # [1024, D]
        m = dict(shared)
        m["condT"] = np.ascontiguousarray(cond[c].T).astype(ml_dtypes.bfloat16)
        m["embT"] = np.ascontiguousarray(emb.T)
        in_maps.append(m)
    return in_maps


def _maybe_install_ntff_hook():
    """If tracing was requested via BASS_TRACE, make sure the axon NTFF hook
    module exists (this image's antenv lacks it); never fatal."""
    import os
    if not os.environ.get("BASS_TRACE"):
        return
    try:
        import antenv.axon_hooks  # noqa: F401
        return
    except ImportError:
        pass
    try:
        import types
        import antenv
        sys.path.insert(0, "/root/.axon_site")
        from trn_agent_boot.trn_boot import _ntff_profile_via_ctypes
        hook = _ntff_profile_via_ctypes("/opt/axon/libaxon_pjrt.so")
        mod = types.ModuleType("antenv.axon_hooks")
        mod._hook = hook
        mod.get_axon_ntff_profile_hook = lambda: mod._hook
        mod.set_axon_ntff_profile_hook = lambda h: setattr(mod, "_hook", h)
        sys.modules["antenv.axon_hooks"] = mod
        antenv.axon_hooks = mod
    except Exception:
        import os
        os.environ["BASS_NEVER_TRACE"] = "1"


def kernel(**inputs) -> np.ndarray:
    global _COMPILED
    _maybe_install_ntff_hook()
    if _COMPILED is None:
        _COMPILED = _build()
    nc = _COMPILED
    in_maps = _prep(inputs)
    res = bass_utils.run_bass_kernel_spmd(nc, in_maps, core_ids=list(range(N_CORES)))
    out = np.stack([res.results[c]["out"].T for c in range(B)], 0)
    kernel._last = res
    return out.astype(np.float32)


# revision 36
# speedup vs baseline: 1.2230x; 1.0328x over previous
"""CrossCondGPTBase forward on 8 Trainium2 NeuronCores.

Strategy: pure data parallelism over batch (B=8 -> 1 sample per core).
Per core, a 12-layer GPT forward over T=1536 tokens, D=768, H=12 heads.

v2: full-bf16 GEMM datapath (weights + activations bf16, fp32 PSUM, fp32
residual/LN stats) — bf16 matmuls avoid fp32r's ap<256 4x penalty and halve
stationary-load time.  Causal masking via a DVE broadcast-multiply on the
exp'd probabilities (the old gpsimd affine_select serialized attention).
Partial score matmuls for the kr=2,3 diagonal offsets are merged into
multi-dim-AP single matmuls within one PSUM bank.  PSUM: 3x sT scratch +
yT [65,3,512] accumulator + 2x rotating projection bank = 8 banks.  The
next head-pair's Q/K projection is emitted between scores and AV so the PE
stays busy while ACT runs the exps.  Evacuations ride on DVE (tensor_scalar
with per-partition bias), keeping ACT for exp/gelu/LN-rsqrt only.
"""

import sys
import numpy as np

sys.path.insert(0, "/opt/trn_rl_repo")

import concourse.bass as bass
import concourse.mybir as mybir
import concourse.tile as tile
from concourse import bacc, bass_utils
import ml_dtypes

F32 = mybir.dt.float32
F32R = mybir.dt.float32r
BF16 = mybir.dt.bfloat16
AF = mybir.ActivationFunctionType
ALU = mybir.AluOpType

L, D, H, BS = 12, 768, 12, 512
NM, B = 438, 8
HD = D // H          # 64
T = 3 * BS           # 1536
KC = D // 128        # 6 feature chunks
TT = T // 128        # 12 token tiles
NT = T // 512        # 3 token groups
FF = 4 * D           # 3072
FC = FF // 128       # 24
EPS = 1e-5
N_CORES = 8
N_LAYERS = L         # knob for compile testing
DEBUG = False

_COMPILED = None


def _build():
    nc = bacc.Bacc("TRN2", target_bir_lowering=False, debug=False, num_devices=1)

    # ---- DRAM tensors ----
    d_condT = nc.dram_tensor("condT", [NM, BS], BF16, kind="ExternalInput")
    d_embT = nc.dram_tensor("embT", [D, 2 * BS], F32R, kind="ExternalInput")
    d_pos0 = nc.dram_tensor("pos0", [D, BS], F32, kind="ExternalInput")
    d_condw = nc.dram_tensor("cond_w", [NM, D], BF16, kind="ExternalInput")
    d_condb = nc.dram_tensor("cond_b", [D], F32, kind="ExternalInput")
    d_wq = nc.dram_tensor("Wq", [L, D, D], BF16, kind="ExternalInput")
    d_wk = nc.dram_tensor("Wk", [L, D, D], BF16, kind="ExternalInput")
    d_wv = nc.dram_tensor("Wv", [L, D, H * 65], BF16, kind="ExternalInput")
    d_wp = nc.dram_tensor("Wp", [L, D, D], BF16, kind="ExternalInput")
    d_w1 = nc.dram_tensor("W1", [L, D, FF], BF16, kind="ExternalInput")
    d_w2 = nc.dram_tensor("W2", [L, FF, D], BF16, kind="ExternalInput")
    d_bq = nc.dram_tensor("bq", [L, D], F32, kind="ExternalInput")
    d_bk = nc.dram_tensor("bk", [L, D], F32, kind="ExternalInput")
    d_bv = nc.dram_tensor("bv", [L, H * 65], BF16, kind="ExternalInput")
    d_bp = nc.dram_tensor("bp", [L, D], F32, kind="ExternalInput")
    d_b1 = nc.dram_tensor("b1", [L, FF], F32, kind="ExternalInput")
    d_b2 = nc.dram_tensor("b2", [L, D], F32, kind="ExternalInput")
    d_sel2 = nc.dram_tensor("sel2", [2, 128], BF16, kind="ExternalInput")
    d_ones1 = nc.dram_tensor("ones1", [1, 128], F32R, kind="ExternalInput")
    d_onesk = nc.dram_tensor("onesk", [128, 1], F32R, kind="ExternalInput")
    d_oneskb = nc.dram_tensor("oneskb", [128, 1], BF16, kind="ExternalInput")
    d_mask3 = nc.dram_tensor("mask3", [128, NT * 128], BF16, kind="ExternalInput")
    d_out = nc.dram_tensor("out", [D, T], F32, kind="ExternalOutput")

    with tile.TileContext(nc) as tc:
        _emit(nc, tc, locals())
    nc.compile()
    return nc


def _emit(nc, tc, d):
    import contextlib

    ctx = contextlib.ExitStack()
    st = ctx.enter_context(tc.tile_pool(name="state", bufs=1))
    qk = ctx.enter_context(tc.tile_pool(name="qk", bufs=2))
    scr = ctx.enter_context(tc.tile_pool(name="scr", bufs=4))
    gp = ctx.enter_context(tc.tile_pool(name="gpool", bufs=8))
    wp_ = ctx.enter_context(tc.tile_pool(name="wpool", bufs=1))
    wmlp = ctx.enter_context(tc.tile_pool(name="wmlp", bufs=2))
    psQ = ctx.enter_context(tc.tile_pool(name="psQ", bufs=2, space="PSUM"))
    psY = ctx.enter_context(tc.tile_pool(name="psY", bufs=1, space="PSUM"))
    psS = ctx.enter_context(tc.tile_pool(name="psS", bufs=3, space="PSUM"))

    # persistent state tiles
    x_t = [st.tile([128, T], F32R, tag=f"x{i}", name=f"x{i}") for i in range(KC)]
    xn_t = [st.tile([128, T], BF16, tag=f"xn{i}", name=f"xn{i}") for i in range(KC)]
    y_t = [st.tile([128, T], BF16, tag=f"y{i}", name=f"y{i}") for i in range(KC)]
    v_t = [st.tile([128, H * 65], BF16, tag=f"v{i}", name=f"v{i}") for i in range(TT)]
    sums = st.tile([H, T], BF16, tag="sums")
    ytmp = st.tile([65, T], BF16, tag="ytmp")
    lnscr1 = st.tile([128, 512], F32R, tag="lnscr1")
    lnscr2 = st.tile([128, 512], F32R, tag="lnscr2")
    sel2_t = st.tile([2, 128], BF16, tag="sel2")
    ones1_t = st.tile([1, 128], F32R, tag="ones1")
    onesk_t = st.tile([128, 1], F32R, tag="onesk")
    oneskb_t = st.tile([128, 1], BF16, tag="oneskb")
    mask3_t = st.tile([128, NT, 128], BF16, tag="mask3")
    eps_t = st.tile([128, 1], F32, tag="eps")
    nc.vector.memset(eps_t, EPS)
    nc.sync.dma_start(out=sel2_t, in_=d["d_sel2"].ap())
    nc.sync.dma_start(out=ones1_t, in_=d["d_ones1"].ap())
    nc.sync.dma_start(out=onesk_t, in_=d["d_onesk"].ap())
    nc.sync.dma_start(out=oneskb_t, in_=d["d_oneskb"].ap())
    nc.sync.dma_start(
        out=mask3_t, in_=d["d_mask3"].ap().rearrange("p (g n) -> p g n", g=NT)
    )

    # ---- init: x = [cond @ cond_w + cond_b + pos | emb(+pos) ] ----
    ct = wp_.tile([128, 4 * BS], BF16, tag="wv", name="ct")
    nc.sync.dma_start(
        out=ct[:, 0 : 3 * BS].rearrange("p (c n) -> p c n", c=3),
        in_=d["d_condT"].ap()[0:384, :].rearrange("(c p) n -> p c n", p=128),
    )
    nc.sync.dma_start(out=ct[0:54, 3 * BS : 4 * BS], in_=d["d_condT"].ap()[384:438, :])
    cw = wp_.tile([128, 4 * D], BF16, tag="wq", name="cw")
    nc.sync.dma_start(
        out=cw[:, 0 : 3 * D].rearrange("p (c n) -> p c n", c=3),
        in_=d["d_condw"].ap()[0:384, :].rearrange("(c p) n -> p c n", p=128),
    )
    nc.sync.dma_start(out=cw[0:54, 3 * D : 4 * D], in_=d["d_condw"].ap()[384:438, :])
    cb_t = st.tile([128, KC], F32, tag="condb")
    nc.sync.dma_start(out=cb_t, in_=d["d_condb"].ap().rearrange("(c p) -> p c", p=128))
    for mc in range(KC):
        nc.gpsimd.dma_start(
            out=x_t[mc][:, 0:BS], in_=d["d_pos0"].ap()[mc * 128 : (mc + 1) * 128, :]
        )
        # embeddings (pos already added on host)
        nc.sync.dma_start(
            out=x_t[mc][:, BS:T], in_=d["d_embT"].ap()[mc * 128 : (mc + 1) * 128, :]
        )
    for mc in range(KC):
        pt = psQ.tile([128, 512], F32, tag="ps", name="ps")
        for kci in range(4):
            kk = 128 if kci < 3 else 54
            nc.tensor.matmul(
                pt,
                cw[0:kk, kci * D + mc * 128 : kci * D + mc * 128 + 128],
                ct[0:kk, kci * BS : (kci + 1) * BS],
                start=(kci == 0),
                stop=(kci == 3),
            )
        # pos segment 0 was DMA'd into x below; add proj + bias on top
        nc.vector.scalar_tensor_tensor(
            out=x_t[mc][:, 0:BS],
            in0=pt,
            scalar=cb_t[:, mc : mc + 1],
            in1=x_t[mc][:, 0:BS],
            op0=ALU.add,
            op1=ALU.add,
        )

    def layernorm_nt(nt):
        """x -> xn = (x - mean) * rsqrt(var + eps), one 512-token group."""
        s1 = psQ.tile([128, 512], F32, tag="ps", name="s1")
        s2 = psQ.tile([128, 512], F32, tag="ps", name="s2")
        sl = slice(nt * 512, (nt + 1) * 512)
        for kc in range(KC):
            nc.tensor.matmul(
                s1[0:1, :], onesk_t, x_t[kc][:, sl],
                start=(kc == 0), stop=(kc == KC - 1),
            )
        for kc in range(KC):  # x^2 scratch in xn tiles (bf16), per nt
            nc.vector.tensor_tensor(
                out=xn_t[kc][:, sl], in0=x_t[kc][:, sl], in1=x_t[kc][:, sl],
                op=ALU.mult,
            )
            nc.tensor.matmul(
                s2[0:1, :], oneskb_t, xn_t[kc][:, sl],
                start=(kc == 0), stop=(kc == KC - 1),
            )
        # evac per-token sums to SBUF (scaled to means)
        nc.scalar.activation(out=lnscr1[0:1, :], in_=s1[0:1, :], func=AF.Identity,
                             scale=1.0 / D)
        nc.scalar.activation(out=lnscr2[0:1, :], in_=s2[0:1, :], func=AF.Identity,
                             scale=1.0 / D)
        m_bc = psQ.tile([128, 512], F32, tag="ps", name="m_bc")
        q_bc = psQ.tile([128, 512], F32, tag="ps", name="q_bc")
        nc.tensor.matmul(m_bc, ones1_t, lnscr1[0:1, :], start=True, stop=True)
        nc.tensor.matmul(q_bc, ones1_t, lnscr2[0:1, :], start=True, stop=True)
        # var = E[x^2] - mean^2  -> lnscr1 ; a = 1/sqrt(var+eps) -> lnscr2
        # (Sqrt on ACT + fast reciprocal on DVE keeps Ln/Exp tables unloaded)
        nc.scalar.activation(out=lnscr1, in_=m_bc, func=AF.Square)
        nc.vector.tensor_tensor(out=lnscr1, in0=q_bc, in1=lnscr1, op=ALU.subtract)
        nc.scalar.activation(out=lnscr2, in_=lnscr1, func=AF.Ln, bias=eps_t)
        nc.scalar.activation(out=lnscr2, in_=lnscr2, func=AF.Exp, scale=-0.5)
        for kc in range(KC):
            nc.vector.tensor_tensor(
                out=xn_t[kc][:, sl], in0=x_t[kc][:, sl], in1=m_bc, op=ALU.subtract
            )
            nc.vector.tensor_tensor(
                out=xn_t[kc][:, sl], in0=xn_t[kc][:, sl], in1=lnscr2, op=ALU.mult
            )

    def load_w_cols(pool, dram, l, c0, c1, tag):
        """Load [D, c0:c1] of a layer weight as [128, KC, c1-c0] bf16."""
        t = pool.tile([128, KC, c1 - c0], BF16, tag=tag, name=tag)
        nc.sync.dma_start(
            out=t, in_=dram.ap()[l][:, c0:c1].rearrange("(c p) n -> p c n", p=128)
        )
        return t

    def load_bias(dram, l, n, tag):
        t = st.tile([128, n // 128], F32, tag=tag, name=tag)
        nc.sync.dma_start(
            out=t, in_=dram.ap()[l].rearrange("(c p) -> p c", p=128)
        )
        return t

    def qkproj_chunk(wq, wk, bq_t, bk_t, q_hp, k_hp, hp, nts):
        """Emit Q/K projection matmuls for token groups `nts` of head pair hp."""
        hc = hp
        for dst, w, b in ((q_hp, wq, bq_t), (k_hp, wk, bk_t)):
            for nt in nts:
                pt = psQ.tile([128, 512], F32, tag="ps", name="qk_ps")
                for kc in range(KC):
                    nc.tensor.matmul(
                        pt,
                        w[:, kc, hc * 128 : (hc + 1) * 128],
                        xn_t[kc][:, nt * 512 : (nt + 1) * 512],
                        start=(kc == 0),
                        stop=(kc == KC - 1),
                    )
                nc.vector.tensor_scalar(
                    out=dst[:, nt * 512 : (nt + 1) * 512], in0=pt,
                    scalar1=b[:, hp : hp + 1], scalar2=None, op0=ALU.add,
                )

    for l in range(N_LAYERS):
        # ============ weights/biases up front (DMA overlaps compute) ======
        wv = load_w_cols(wp_, d["d_wv"], l, 0, H * 65, "wv")
        bv_bc = st.tile([128, H * 65], BF16, tag="bvbc")
        nc.gpsimd.dma_start(
            out=bv_bc,
            in_=bass.AP(
                tensor=d["d_bv"].ap().tensor,
                offset=l * H * 65,
                ap=[[0, 128], [1, H * 65]],
            ),
        )
        bq_t = load_bias(d["d_bq"], l, D, "bq")
        bk_t = load_bias(d["d_bk"], l, D, "bk")
        bp_t = load_bias(d["d_bp"], l, D, "bp")
        wq = load_w_cols(wp_, d["d_wq"], l, 0, D, "wq")
        wk = load_w_cols(wp_, d["d_wk"], l, 0, D, "wk")

        # ============ LN1 woven with V projection (token-major) ===========
        for nt in range(NT):
            layernorm_nt(nt)
            for tt in range(4 * nt, 4 * nt + 4):
                pva = psS.tile([128, 512], F32, tag="sT", name="pva")
                pvb = psS.tile([128, 512], F32, tag="sT", name="pvb")
                for kc in range(KC):
                    lh = xn_t[kc][:, tt * 128 : (tt + 1) * 128]
                    nc.tensor.matmul(pva, lh, wv[:, kc, 0:512],
                                     start=(kc == 0), stop=(kc == KC - 1))
                    nc.tensor.matmul(pvb[:, 0:268], lh, wv[:, kc, 512 : H * 65],
                                     start=(kc == 0), stop=(kc == KC - 1))
                nc.vector.tensor_tensor(out=v_t[tt][:, 0:512], in0=pva,
                                        in1=bv_bc[:, 0:512], op=ALU.add)
                nc.vector.tensor_tensor(out=v_t[tt][:, 512:780], in0=pvb[:, 0:268],
                                        in1=bv_bc[:, 512:780], op=ALU.add)

        # ============ attention: per head pair project q,k then attend ====
        qh = [qk.tile([128, T], BF16, tag="q", name=f"q{hp}") for hp in range(KC)]
        kh = [qk.tile([128, T], BF16, tag="k", name=f"k{hp}") for hp in range(KC)]
        qkproj_chunk(wq, wk, bq_t, bk_t, qh[0], kh[0], 0, range(NT))
        for hp in range(KC):
            q_hp, k_hp = qh[hp], kh[hp]
            yT = psY.tile([65, NT, 512], F32, tag="yT", name="yT")
            for hh in range(2):
                h = 2 * hp + hh
                hs = slice(64 * hh, 64 * hh + 64)
                q3 = q_hp[hs, :].rearrange("p (g n) -> p g n", g=NT)
                pT = [None] * TT

                def scores_one(kt):
                    kr = kt % 4
                    lo = 128 * kr
                    w = 512 - lo
                    pT[kt] = scr.tile([128, NT, 512], BF16, tag="pT", bufs=5,
                                      name="pT")
                    kst = k_hp[hs, kt * 128 : (kt + 1) * 128]
                    if kr == 3:
                        sT = psS.tile([128, 512], F32, tag="sT", name="sT")
                        nc.tensor.matmul(
                            sT[:, 0 : NT * w].rearrange("p (g n) -> p g n", g=NT),
                            kst, q3[:, :, lo:512], start=True, stop=True,
                        )
                        nc.scalar.activation(
                            out=pT[kt][:, :, lo:512],
                            in_=sT[:, 0 : NT * w].rearrange("p (g n) -> p g n", g=NT),
                            func=AF.Exp,
                        )
                    elif kr == 2:
                        sT = psS.tile([128, 512], F32, tag="sT", name="sT")
                        nc.tensor.matmul(
                            sT.rearrange("p (g n) -> p g n", g=2),
                            kst, q3[:, 0:2, lo:512], start=True, stop=True,
                        )
                        sT2 = psS.tile([128, 512], F32, tag="sT", name="sT")
                        nc.tensor.matmul(
                            sT2[:, 0:w], kst, q_hp[hs, 2 * 512 + lo : 3 * 512],
                            start=True, stop=True,
                        )
                        nc.scalar.activation(
                            out=pT[kt][:, 0:2, lo:512],
                            in_=sT.rearrange("p (g n) -> p g n", g=2),
                            func=AF.Exp,
                        )
                        nc.scalar.activation(
                            out=pT[kt][:, 2, lo:512], in_=sT2[:, 0:w], func=AF.Exp,
                        )
                    else:
                        for qb in range(NT):
                            sT = psS.tile([128, 512], F32, tag="sT", name="sT")
                            nc.tensor.matmul(
                                sT[:, 0:w], kst,
                                q_hp[hs, qb * 512 + lo : (qb + 1) * 512],
                                start=True, stop=True,
                            )
                            nc.scalar.activation(
                                out=pT[kt][:, qb, lo:512], in_=sT[:, 0:w],
                                func=AF.Exp,
                            )
                    nc.vector.tensor_tensor(
                        out=pT[kt][:, :, lo : lo + 128],
                        in0=pT[kt][:, :, lo : lo + 128],
                        in1=mask3_t,
                        op=ALU.mult,
                    )

                def av_one(kt):
                    lo = 128 * (kt % 4)
                    for qb in range(NT):
                        nc.tensor.matmul(
                            yT[:, qb, lo:512],
                            v_t[kt][:, h * 65 : h * 65 + 65],
                            pT[kt][:, qb, lo:512],
                            start=(kt == 0),
                            stop=(kt == TT - 1),
                        )

                # kt groups of 3: scores(g) -> PE filler -> AV(g) so the PE
                # streams QK projections of the next pair while ACT exps run.
                filler = []
                if hp + 1 < KC:
                    nxt = [hh] if hh == 0 else [1, 2]
                    filler = [(qh[hp + 1], wq, bq_t), (kh[hp + 1], wk, bk_t)]
                    filler = [(dst, w_, b_, nt) for nt in nxt
                              for (dst, w_, b_) in filler]
                fi = 0
                for grp in range(4):
                    kts = range(3 * grp, 3 * grp + 3)
                    for kt in kts:
                        scores_one(kt)
                    # one QK projection chain as PE filler per group
                    if fi < len(filler):
                        dst, w_, b_, nt = filler[fi]
                        fi += 1
                        pt = psQ.tile([128, 512], F32, tag="ps", name="qk_ps")
                        for kc in range(KC):
                            nc.tensor.matmul(
                                pt,
                                w_[:, kc, (hp + 1) * 128 : (hp + 2) * 128],
                                xn_t[kc][:, nt * 512 : (nt + 1) * 512],
                                start=(kc == 0),
                                stop=(kc == KC - 1),
                            )
                        nc.vector.tensor_scalar_add(
                            out=dst[:, nt * 512 : (nt + 1) * 512], in0=pt,
                            scalar1=b_[:, hp + 1 : hp + 2],
                        )
                    for kt in kts:
                        av_one(kt)
                while fi < len(filler):
                    dst, w_, b_, nt = filler[fi]
                    fi += 1
                    pt = psQ.tile([128, 512], F32, tag="ps", name="qk_ps")
                    for kc in range(KC):
                        nc.tensor.matmul(
                            pt,
                            w_[:, kc, (hp + 1) * 128 : (hp + 2) * 128],
                            xn_t[kc][:, nt * 512 : (nt + 1) * 512],
                            start=(kc == 0),
                            stop=(kc == KC - 1),
                        )
                    nc.vector.tensor_scalar_add(
                        out=dst[:, nt * 512 : (nt + 1) * 512], in0=pt,
                        scalar1=b_[:, hp + 1 : hp + 2],
                    )
                # evac: y rows + sums row (bf16); partition-shift via DMA
                nc.vector.tensor_scalar_mul(
                    out=ytmp, in0=yT.rearrange("p g n -> p (g n)"), scalar1=1.0
                )
                nc.sync.dma_start(
                    out=y_t[hp][64 * hh : 64 * hh + 64, :], in_=ytmp[0:64, :]
                )
                nc.sync.dma_start(out=sums[h : h + 1, :], in_=ytmp[64:65, :])

        # ====== per-nt: y-norm -> proj; then LN2; then MLP ======
        nc.scalar.activation(out=sums, in_=sums, func=AF.Ln)
        nc.scalar.activation(out=sums, in_=sums, func=AF.Exp, scale=-1.0)
        wp = load_w_cols(wp_, d["d_wp"], l, 0, D, "wv")
        b1_t = load_bias(d["d_b1"], l, FF, "b1")
        b2_t = load_bias(d["d_b2"], l, D, "b2")
        for nt in range(NT):
            sl = slice(nt * 512, (nt + 1) * 512)
            for kc in range(KC):
                rstage = scr.tile([2, 512], BF16, tag="rstage", bufs=2,
                                  name="rstage")
                nc.sync.dma_start(
                    out=rstage,
                    in_=sums[2 * kc : 2 * kc + 2, nt * 512 : (nt + 1) * 512],
                )
                rb = psQ.tile([128, 512], F32, tag="ps", name="rb")
                nc.tensor.matmul(rb, sel2_t, rstage, start=True, stop=True)
                nc.vector.tensor_tensor(
                    out=y_t[kc][:, sl], in0=y_t[kc][:, sl], in1=rb, op=ALU.mult,
                )
            for mc in range(KC):
                pt = psQ.tile([128, 512], F32, tag="ps", name="pj_ps")
                for kc in range(KC):
                    nc.tensor.matmul(
                        pt,
                        wp[:, kc, mc * 128 : (mc + 1) * 128],
                        y_t[kc][:, sl],
                        start=(kc == 0),
                        stop=(kc == KC - 1),
                    )
                nc.vector.scalar_tensor_tensor(
                    out=x_t[mc][:, sl],
                    in0=pt,
                    scalar=bp_t[:, mc : mc + 1],
                    in1=x_t[mc][:, sl],
                    op0=ALU.add,
                    op1=ALU.add,
                )
            layernorm_nt(nt)
            gt = [gp.tile([128, NT, 512], BF16, tag="g", name="g") for _ in range(8)]
            for gh in range(8):  # gc groups of 3
                w1 = load_w_cols(wmlp, d["d_w1"], l, gh * 384, (gh + 1) * 384, "w1")
                for gi in range(3):
                    gc = gh * 3 + gi
                    pt = psQ.tile([128, 512], F32, tag="ps", name="g_ps")
                    for kc in range(KC):
                        nc.tensor.matmul(
                            pt, w1[:, kc, gi * 128 : (gi + 1) * 128], xn_t[kc][:, sl],
                            start=(kc == 0), stop=(kc == KC - 1),
                        )
                    nc.scalar.activation(
                        out=gt[gh][:, gi, :], in_=pt, func=AF.Gelu,
                        bias=b1_t[:, gc : gc + 1],
                    )
            for mc in range(KC):
                w2 = wmlp.tile([128, FC, 128], BF16, tag="w2", name="w2")
                nc.sync.dma_start(
                    out=w2,
                    in_=d["d_w2"].ap()[l][:, mc * 128 : (mc + 1) * 128].rearrange(
                        "(c p) n -> p c n", p=128
                    ),
                )
                pt = psQ.tile([128, 512], F32, tag="ps", name="f2_ps")
                for kc in range(FC):
                    nc.tensor.matmul(
                        pt, w2[:, kc, :], gt[kc // 3][:, kc % 3, :],
                        start=(kc == 0), stop=(kc == FC - 1),
                    )
                nc.vector.scalar_tensor_tensor(
                    out=x_t[mc][:, sl],
                    in0=pt,
                    scalar=b2_t[:, mc : mc + 1],
                    in1=x_t[mc][:, sl],
                    op0=ALU.add,
                    op1=ALU.add,
                )

    # ---- output ----
    for mc in range(KC):
        nc.gpsimd.dma_start(
            out=d["d_out"].ap()[mc * 128 : (mc + 1) * 128, :], in_=x_t[mc]
        )
    ctx.close()


def _prep(inputs):
    """Host-side: fold LN params & scales into weights, gather embeddings."""
    f = lambda a: np.asarray(a, dtype=np.float32)
    bf = lambda a: np.asarray(a, dtype=np.float32).astype(ml_dtypes.bfloat16)
    idx_up = np.asarray(inputs["idx_up"]).astype(np.int64)
    idx_down = np.asarray(inputs["idx_down"]).astype(np.int64)
    cond = f(inputs["cond"])
    teu, ted = f(inputs["tok_emb_up"]), f(inputs["tok_emb_down"])
    pos = f(inputs["pos_emb"])[0]  # [3*BS, D]
    ln1w, ln1b = f(inputs["ln1_w"]), f(inputs["ln1_b"])
    ln2w, ln2b = f(inputs["ln2_w"]), f(inputs["ln2_b"])
    Wq, bq = f(inputs["Wq"]), f(inputs["bq"])
    Wk, bk = f(inputs["Wk"]), f(inputs["bk"])
    Wv, bv = f(inputs["Wv"]), f(inputs["bv"])
    Wp, bp = f(inputs["Wp"]), f(inputs["bp"])
    W1, b1 = f(inputs["W1"]), f(inputs["b1"])
    W2, b2 = f(inputs["W2"]), f(inputs["b2"])

    sc = 1.0 / np.sqrt(HD)
    Wq_e = np.einsum("ld,ldm->ldm", ln1w, Wq) * sc
    bq_e = (np.einsum("ld,ldm->lm", ln1b, Wq) + bq) * sc
    Wk_e = np.einsum("ld,ldm->ldm", ln1w, Wk)
    bk_e = np.einsum("ld,ldm->lm", ln1b, Wk) + bk
    Wv_e = np.einsum("ld,ldm->ldm", ln1w, Wv)
    bv_e = np.einsum("ld,ldm->lm", ln1b, Wv) + bv
    W1_e = np.einsum("ld,ldm->ldm", ln2w, W1)
    b1_e = np.einsum("ld,ldm->lm", ln2b, W1) + b1

    # pad V weights/bias: per head 64 value cols + 1 ones col (bias=1, W=0)
    Wv_pad = np.zeros((L, D, H * 65), np.float32)
    bv_pad = np.zeros((L, H * 65), np.float32)
    for h in range(H):
        Wv_pad[:, :, h * 65 : h * 65 + 64] = Wv_e[:, :, h * 64 : (h + 1) * 64]
        bv_pad[:, h * 65 : h * 65 + 64] = bv_e[:, h * 64 : (h + 1) * 64]
        bv_pad[:, h * 65 + 64] = 1.0
    sel2 = np.zeros((2, 128), np.float32)
    sel2[0, 0:64] = 1.0
    sel2[1, 64:128] = 1.0
    # per-128-block causal mask, keep col >= row, replicated x3
    m1 = (np.arange(128)[None, :] >= np.arange(128)[:, None]).astype(np.float32)
    mask3 = np.tile(m1, (1, NT))

    shared = {
        "cond_w": bf(inputs["cond_w"]),
        "cond_b": f(inputs["cond_b"]),
        "pos0": np.ascontiguousarray(pos[:BS].T),
        "Wq": bf(Wq_e), "bq": bq_e, "Wk": bf(Wk_e), "bk": bk_e,
        "Wv": bf(Wv_pad), "bv": bv_pad.astype(ml_dtypes.bfloat16),
        "Wp": bf(Wp), "bp": bp,
        "W1": bf(W1_e), "b1": b1_e,
        "W2": bf(W2), "b2": b2,
        "sel2": sel2.astype(ml_dtypes.bfloat16),
        "ones1": np.ones((1, 128), np.float32),
        "onesk": np.ones((128, 1), np.float32),
        "oneskb": np.ones((128, 1), ml_dtypes.bfloat16),
        "mask3": mask3.astype(ml_dtypes.bfloat16),
    }
    in_maps = []
    for c in range(B):
        emb = np.concatenate(
            [teu[idx_up[c]] + pos[BS : 2 * BS], ted[idx_down[c]] + pos[2 * BS :]], 0
        )  # [1024, D]
        m = dict(shared)
        m["condT"] = np.ascontiguousarray(cond[c].T).astype(ml_dtypes.bfloat16)
        m["embT"] = np.ascontiguousarray(emb.T)
        in_maps.append(m)
    return in_maps


def _maybe_install_ntff_hook():
    """If tracing was requested via BASS_TRACE, make sure the axon NTFF hook
    module exists (this image's antenv lacks it); never fatal."""
    import os
    if not os.environ.get("BASS_TRACE"):
        return
    try:
        import antenv.axon_hooks  # noqa: F401
        return
    except ImportError:
        pass
    try:
        import types
        import antenv
        sys.path.insert(0, "/root/.axon_site")
        from trn_agent_boot.trn_boot import _ntff_profile_via_ctypes
        hook = _ntff_profile_via_ctypes("/opt/axon/libaxon_pjrt.so")
        mod = types.ModuleType("antenv.axon_hooks")
        mod._hook = hook
        mod.get_axon_ntff_profile_hook = lambda: mod._hook
        mod.set_axon_ntff_profile_hook = lambda h: setattr(mod, "_hook", h)
        sys.modules["antenv.axon_hooks"] = mod
        antenv.axon_hooks = mod
    except Exception:
        import os
        os.environ["BASS_NEVER_TRACE"] = "1"


def kernel(**inputs) -> np.ndarray:
    global _COMPILED
    _maybe_install_ntff_hook()
    if _COMPILED is None:
        _COMPILED = _build()
    nc = _COMPILED
    in_maps = _prep(inputs)
    res = bass_utils.run_bass_kernel_spmd(nc, in_maps, core_ids=list(range(N_CORES)))
    out = np.stack([res.results[c]["out"].T for c in range(B)], 0)
    kernel._last = res
    return out.astype(np.float32)
